# revision 43
# baseline (speedup 1.0000x reference)
"""Trainium2 Bass kernel for per-variable-MLP GNN message passing.

Model (reference):
    adj  = ones(D,D) - eye(D)                       # var t cannot see itself
    h0   = leaky_relu(einsum('tij,bj->bti', w0*adjmask, x) + b0)
    h1   = leaky_relu(einsum('tij,btj->bti', w1, h0) + b1)
    out  = einsum('tij,btj->bti', w2, h1) + b2      # (B, D, O)

Sharding: the variable axis t (128) is split across 8 cores (16 vars each);
each core sees the full batch.  Vars are processed in pairs: a pair's two
(64 x K) weight matrices are stacked/block-diagonalized to fill the 128-wide
tensor-engine array; activations live transposed (feature-on-partition,
batch-on-free).

Structure (per core):
- Batch is walked in superchunks of 2x512.  L0 and L1 for one pair each
  write a 2-bank PSUM tile (128, 1024) via two N=512 matmuls; the
  bias+leaky epilogue then runs as ONE FD=1024 op, amortizing the fixed
  per-op overhead of ScalarE/VectorE (the PSUM->SBUF evacuation engines
  are co-bottlenecks with the PE: all three sit at ~100us busy).  The
  six z-banks form one shared 3-deep double-bank rotation.
- L2 packs FOUR chunks into one PSUM bank: chunk c lands in 32-partition
  column-group c%4 (one M=32 zero-padded matmul per pair; each chunk is
  its own accumulation group — the has_written clear of start=True is
  scoped to the written region, NOT the whole bank).  One bias-add
  evacuates 512 batch x 32 outputs per op, and the output DMA is fully
  dense ((128, 2048) fp32 per core = all useful).
- Epilogue work is split between ScalarE (fused Prelu-with-bias, one op
  per tile) and VectorE (bias-add + scalar_tensor_tensor leaky, two ops
  per tile), ratio tuned via the KIND tables.  GpSimd cannot run any
  elementwise op in this toolchain (walrus engine check rejects Pool).

Matmuls run in fp16 (1 cycle/row on the PE, fp32 accumulate in PSUM;
fp8 L2 was tested and fails the 2e-2 gate at 5e-2).  LDWEIGHTS is
hidden behind matmul streaming by the PE reorder window; steady-state
matmul issue rate is ~227ns for N=512 (floor is 216).
"""

import numpy as np

import concourse.bass as bass
import concourse.mybir as mybir
import concourse.tile as tile
from concourse import bacc, bass_utils

F32 = mybir.dt.float32
DT = mybir.dt.float16
NPDT = np.float16

B = 8192  # batch
D = 128  # num variables (t)
H = 64  # hidden
O = 2  # output dim per variable
NCORES = 8
TPC = D // NCORES  # vars per core = 16
NPAIR = TPC // 2  # 8
CH = 512  # batch chunk (psum bank = 512 fp32)
NCHUNK = B // CH  # 16
NSC = NCHUNK // 2  # superchunks of 2 chunks
ALPHA = 0.01  # leaky_relu slope

# If False, bacc's move_matmul_waits_to_ldweights pass is disabled so
# LDWEIGHTS stays wait-free and the PE reorder window can pull it ahead
# of in-flight matmuls.
MOVE_WAITS = True
Z0_DOUBLE = True  # L0 into 2-bank PSUM tiles + FD=1024 epilogues
Z1_DOUBLE = True  # L1 into 2-bank PSUM tiles (shared rotation with z0)

# Epilogue kinds: "act" = ScalarE fused Prelu; "dv" = VectorE bias-add +
# VectorE leaky.  GpSimd cannot run any elementwise op in this
# toolchain (walrus engine check).
L0_KINDS = ["act"] * 8  # one per pair, FD=1024 ops
# All-FD1024 balance: ACT 90 of 128 tiles, DVE 38 (2-op is ~2.4x/tile)
L1_KINDS = ["act" if i % 16 in (0, 1, 3, 4, 6, 9, 11, 12, 14) else "dv"
            for i in range(16)]  # per (2p+ci); FD=512 mode only
# FD1024 mode: per (sc, p) -> 64 slots; 27 act / 37 dv (Bresenham spread;
# 26 and 28 measured worse, as did moving dv tiles into L0 and
# concentrating the last superchunk's tiles on ScalarE)
L1D_KINDS = ["act" if (j * 27) // 64 != ((j + 1) * 27) // 64 else "dv"
             for j in range(64)]

Prelu = mybir.ActivationFunctionType.Prelu
MULT = mybir.AluOpType.mult
MAX = mybir.AluOpType.max


class _Bacc(bacc.Bacc):
    def move_matmul_waits_to_ldweights(self):
        if MOVE_WAITS:
            super().move_matmul_waits_to_ldweights()


def _build_program():
    nc = _Bacc(trn_type="TRN2")

    xt = nc.dram_tensor("xt", (D, B), DT, kind="ExternalInput")
    w0t = nc.dram_tensor("w0t", (D, NPAIR * 128), DT, kind="ExternalInput")
    w1bd = nc.dram_tensor("w1bd", (128, NPAIR * 128), DT, kind="ExternalInput")
    # w2g: per pair p an M=32 stationary (full col-group layout, pair p's
    # 4 cols at 4p..4p+4, rest zero)
    w2g = nc.dram_tensor("w2g", (128, NPAIR * 32), DT, kind="ExternalInput")
    b0c = nc.dram_tensor("b0c", (128, NPAIR), F32, kind="ExternalInput")
    b1c = nc.dram_tensor("b1c", (128, NPAIR), F32, kind="ExternalInput")
    # b2c: dense per-partition bias, partition 32g+4p+2v+o = b2[2p+v, o]
    b2c = nc.dram_tensor("b2c", (128, 1), F32, kind="ExternalInput")
    # dense output: partition 32*(c%4)+4p+2v+o, column (c//4)*512 + (b%512)
    otA = nc.dram_tensor("otA", (128, B // 4), F32, kind="ExternalOutput")

    with tile.TileContext(nc) as tc:
        with (
            tc.tile_pool(name="wp", bufs=1) as wp,
            tc.tile_pool(name="hp", bufs=2) as hp,
            tc.tile_pool(name="op", bufs=4) as op,
            tc.tile_pool(name="z0p", bufs=3 if Z1_DOUBLE else 2,
                         space="PSUM") as z0p,
            tc.tile_pool(name="z1p", bufs=1, space="PSUM") as z1p,
            tc.tile_pool(name="z2p", bufs=2 if Z1_DOUBLE else 1,
                         space="PSUM") as z2p,
        ):
            xs = wp.tile([D, B], DT)
            w0s = wp.tile([D, NPAIR * 128], DT)
            w1s = wp.tile([128, NPAIR * 128], DT)
            w2s = wp.tile([128, NPAIR * 32], DT)
            b0s = wp.tile([128, NPAIR], F32)
            b1s = wp.tile([128, NPAIR], F32)
            b2s = wp.tile([128, 1], F32)
            nc.sync.dma_start(xs[:, 0 : 2 * CH], xt[:, 0 : 2 * CH])
            nc.sync.dma_start(w0s[:], w0t[:])
            nc.sync.dma_start(b0s[:], b0c[:])
            nc.sync.dma_start(w1s[:], w1bd[:])
            nc.sync.dma_start(b1s[:], b1c[:])
            nc.sync.dma_start(w2s[:], w2g[:])
            nc.sync.dma_start(b2s[:], b2c[:])
            xs_loaded = 1  # superchunks loaded

            def leaky_epilogue(dst, z, bias_col, kind, fd):
                """dst (fp16 SBUF) = leaky_relu(z + bias), z in PSUM."""
                if kind == "act":
                    nc.scalar.activation(
                        dst[:], z[:], Prelu, bias=bias_col, scale=1.0, alpha=ALPHA
                    )
                elif kind == "dg3":
                    # 3-op pipe: DVE bias-add, DVE alpha-scale (fast mode),
                    # GpSimd tensor-tensor max (stt is not legal on Pool)
                    y = hp.tile([128, fd], DT, tag=f"y{fd}", name=f"y{fd}", bufs=12)
                    u = hp.tile([128, fd], DT, tag=f"u{fd}", name=f"u{fd}", bufs=8)
                    nc.vector.tensor_scalar_add(y[:], z[:], bias_col)
                    nc.vector.tensor_scalar_mul(u[:], y[:], ALPHA)
                    nc.gpsimd.tensor_max(dst[:], y[:], u[:])
                else:
                    y = hp.tile([128, fd], DT, tag=f"y{fd}", name=f"y{fd}", bufs=12)
                    nc.vector.tensor_scalar_add(y[:], z[:], bias_col)
                    nc.vector.scalar_tensor_tensor(dst[:], y[:], ALPHA, y[:], MULT, MAX)

            # PE warmup: dummy matmuls with no dependencies at all (operands
            # are an uninitialized scratch tile; the psum result is never
            # read) so the HAM clock-gate reaches 8/8 while input DMAs run.
            warm = wp.tile([128, CH], DT, name="warm")
            nc.vector.memset(warm[:], 0.0)
            # 3 warm MMs: enough to cover until the input DMAs land (~9.2us);
            # the HAM window is wall-time based, so the first few real MMs
            # finish the ramp — cheaper than over-covering with 8
            for _ in range(3):
                wpool = z2p if Z1_DOUBLE else z1p
                wps = wpool.tile([128, CH], F32, name="warmps",
                                 tag="z2" if Z1_DOUBLE else "z1")
                nc.tensor.matmul(wps[:], warm[:, 0:128], warm[:], start=True,
                                 stop=True)

            h0_tiles = [None] * NSC  # [sc] -> list of 8 (128,1024) tiles
            h1_tiles = [None] * NSC  # [sc] -> [ci][p] (128,512) tiles
            z2_tile = [None]  # current 4-chunk output bank

            for s in range(NSC + 2):
                # prefetch next superchunk's x
                if s + 1 < NSC and xs_loaded <= s + 1:
                    lo = (s + 1) * 2 * CH
                    nc.sync.dma_start(xs[:, lo : lo + 2 * CH],
                                      xt[:, lo : lo + 2 * CH])
                    xs_loaded = s + 2

                a_tiles = [] if s < NSC else None
                b_tiles = [[None] * NPAIR, [None] * NPAIR] if 1 <= s <= NSC else None

                for p in range(NPAIR):
                    # ---- stage A: L0 (superchunk s), FD=1024 epilogue ----
                    if s < NSC:
                        if Z0_DOUBLE:
                            z0 = z0p.tile([128, 2 * CH], F32, tag="z0", name="z0")
                            for ci in range(2):
                                cs = bass.ts(2 * s + ci, CH)
                                nc.tensor.matmul(
                                    z0[:, bass.ts(ci, CH)],
                                    w0s[:, bass.ts(p, 128)],
                                    xs[:, cs],
                                    start=True, stop=True,
                                )
                            h0 = hp.tile([128, 2 * CH], DT, tag="h0",
                                         name=f"h0_{s}_{p}", bufs=16)
                            leaky_epilogue(h0, z0, b0s[:, p : p + 1],
                                           L0_KINDS[p], 2 * CH)
                        else:
                            h0 = hp.tile([128, 2 * CH], DT, tag="h0",
                                         name=f"h0_{s}_{p}", bufs=16)
                            for ci in range(2):
                                cs = bass.ts(2 * s + ci, CH)
                                z0 = z0p.tile([128, CH], F32, tag="z0",
                                              name="z0", bufs=4)
                                nc.tensor.matmul(
                                    z0[:], w0s[:, bass.ts(p, 128)], xs[:, cs],
                                    start=True, stop=True,
                                )
                                leaky_epilogue(h0[:, bass.ts(ci, CH)], z0,
                                               b0s[:, p : p + 1],
                                               L0_KINDS[p], CH)
                        a_tiles.append(h0)

                    # ---- stage B: L1 (superchunk s-1) ----
                    if 1 <= s <= NSC:
                        sp = s - 1
                        h0d = h0_tiles[sp][p]
                        if Z1_DOUBLE:
                            z1 = z0p.tile([128, 2 * CH], F32, tag="z0",
                                          name=f"z1_{sp}_{p}")
                            for ci in range(2):
                                nc.tensor.matmul(
                                    z1[:, bass.ts(ci, CH)],
                                    w1s[:, bass.ts(p, 128)],
                                    h0d[:, bass.ts(ci, CH)],
                                    start=True, stop=True,
                                )
                            h1d = hp.tile([128, 2 * CH], DT, tag="h1d",
                                          name=f"h1_{sp}_{p}", bufs=24)
                            leaky_epilogue(h1d, z1, b1s[:, p : p + 1],
                                           L1D_KINDS[sp * NPAIR + p], 2 * CH)
                            for ci in range(2):
                                b_tiles[ci][p] = h1d[:, bass.ts(ci, CH)]
                        else:
                            for ci in range(2):
                                z1 = z1p.tile([128, CH], F32, tag="z1",
                                              name=f"z1_{sp}_{p}_{ci}")
                                nc.tensor.matmul(
                                    z1[:], w1s[:, bass.ts(p, 128)],
                                    h0d[:, bass.ts(ci, CH)],
                                    start=True, stop=True,
                                )
                                h1 = hp.tile([128, CH], DT, tag="h1",
                                             name=f"h1_{sp}_{p}_{ci}", bufs=36)
                                leaky_epilogue(h1, z1, b1s[:, p : p + 1],
                                               L1_KINDS[2 * p + ci], CH)
                                b_tiles[ci][p] = h1

                    # ---- stage C: L2 (superchunk s-2), 4-chunk PSUM pack ----
                    if 2 <= s:
                        sq = s - 2
                        if sq % 2 == 0 and p == 0:
                            z2_tile[0] = z2p.tile([128, CH], F32, tag="z2",
                                                  name=f"z2_{sq // 2}")
                        z2 = z2_tile[0]
                        for ci in range(2):
                            c = 2 * sq + ci
                            g = c % 4
                            # has_written clear is scoped to the region the
                            # matmul writes, so each chunk (col-group) is its
                            # own accumulation group
                            nc.tensor.matmul(
                                z2[32 * g : 32 * g + 32, :],
                                w2s[:, bass.ts(p, 32)],
                                h1_tiles[sq][ci][p][:],
                                start=(p == 0),
                                stop=(p == NPAIR - 1),
                                tile_position=(0, 32 * g),
                            )

                if s < NSC:
                    h0_tiles[s] = a_tiles
                if 1 <= s <= NSC:
                    h1_tiles[s - 1] = b_tiles
                    # h0 of superchunk s-1 fully consumed

                # ---- stage C tail: evacuate + store a filled z2 bank ----
                if 2 <= s and (s - 2) % 2 == 1:
                    g4 = (s - 2) // 2
                    ob = op.tile([128, CH], F32, tag="ob", name=f"ob_{g4}")
                    nc.vector.tensor_scalar_add(ob[:], z2_tile[0][:], b2s[:, 0:1])
                    nc.sync.dma_start(otA[:, bass.ts(g4, CH)], ob[:])
                    h1_tiles[s - 2] = None

    nc.finalize()
    return nc


_prog = None


def _get_program():
    global _prog
    if _prog is None:
        _prog = _build_program()
    return _prog


def _shard_inputs(x, w0, w1, w2, b0, b1, b2):
    """Host-side relayout + t-sharding. Returns list of 8 in_maps."""
    x = np.asarray(x, np.float32)
    w0 = np.array(w0, np.float32)  # copy: we zero the adjacency diagonal
    w1 = np.asarray(w1, np.float32)
    w2 = np.asarray(w2, np.float32)
    b0 = np.asarray(b0, np.float32)
    b1 = np.asarray(b1, np.float32)
    b2 = np.asarray(b2, np.float32)

    # adjacency mask: variable t cannot see itself -> w0[t, :, t] = 0
    ar = np.arange(D)
    w0[ar, :, ar] = 0.0

    xt = np.ascontiguousarray(x.T).astype(NPDT)  # (128, 8192)

    in_maps = []
    for c in range(NCORES):
        ts_ = slice(c * TPC, (c + 1) * TPC)
        w0c, w1c, w2c = w0[ts_], w1[ts_], w2[ts_]
        b0cc, b1cc, b2cc = b0[ts_], b1[ts_], b2[ts_]

        # w0t: (128 j, pair*128 + [ta's 64 i | tb's 64 i])
        w0T = w0c.transpose(0, 2, 1)  # (16, 128 j, 64 i)
        w0t_ = np.ascontiguousarray(
            w0T.reshape(NPAIR, 2, D, H).transpose(2, 0, 1, 3).reshape(D, NPAIR * 128)
        ).astype(NPDT)

        # w1bd: per-pair 128x128 block-diagonal (2 vars stacked)
        bd1 = np.zeros((NPAIR, 128, 128), np.float32)
        for p in range(NPAIR):
            bd1[p, 0:H, 0:H] = w1c[2 * p].T
            bd1[p, H:128, H:128] = w1c[2 * p + 1].T
        w1bd_ = np.ascontiguousarray(
            bd1.transpose(1, 0, 2).reshape(128, NPAIR * 128)
        ).astype(NPDT)

        # w2g: per-pair (128 K, 32 M) stationary; within the 32-wide
        # col-group, pair p's slots are cols 4p+2v+o; all other cols zero.
        g2 = np.zeros((NPAIR, 128, 32), np.float32)
        for p in range(NPAIR):
            for v in range(2):
                t = 2 * p + v
                for o in range(O):
                    g2[p, v * H : (v + 1) * H, 4 * p + 2 * v + o] = w2c[t][o]
        w2g_ = np.ascontiguousarray(
            g2.transpose(1, 0, 2).reshape(128, NPAIR * 32)
        ).astype(NPDT)

        b0c_ = np.ascontiguousarray(b0cc.reshape(NPAIR, 128).T).astype(np.float32)
        b1c_ = np.ascontiguousarray(b1cc.reshape(NPAIR, 128).T).astype(np.float32)

        # b2 dense: partition 32g + 4p + 2v + o = b2[2p+v, o] (same per g)
        pat = b2cc.reshape(NPAIR * 2 * O)  # [4p+2v+o]
        b2c_ = np.tile(pat, 4).reshape(128, 1).astype(np.float32)

        in_maps.append(
            {
                "xt": xt,
                "w0t": w0t_,
                "w1bd": w1bd_,
                "w2g": w2g_,
                "b0c": b0c_,
                "b1c": b1c_,
                "b2c": b2c_,
            }
        )
    return in_maps


def _unshard_outputs(results):
    out = np.empty((B, D, O), np.float32)
    for c in range(NCORES):
        ot = results[c]["otA"]  # (128, 2048)
        # partition = 32g + 4p + 2v + o ; column = g4*512 + j
        blk = (
            ot.reshape(4, NPAIR, 2, O, 4, CH)  # (g, p, v, o, g4, j)
            .transpose(4, 0, 5, 1, 2, 3)  # (g4, g, j, p, v, o)
            .reshape(B, TPC, O)
        )
        out[:, c * TPC : (c + 1) * TPC, :] = blk
    return out


def kernel(x, w0, w1, w2, b0, b1, b2):
    nc = _get_program()
    in_maps = _shard_inputs(x, w0, w1, w2, b0, b1, b2)
    res = bass_utils.run_bass_kernel_spmd(nc, in_maps, core_ids=list(range(NCORES)))
    return _unshard_outputs(res.results)
